# revision 1
# baseline (speedup 1.0000x reference)
"""Trainium2 Bass kernel for BlockChunkedActivityRoutedNet.

Reference computation (B=4096, IN_F=4096, 8 chunks of 512, top-2 by mean|x|,
chunk-expert Linears 512->512, concat -> final Linear 1024->4096):

    xr = x.reshape(B, 8, 512)
    activities = mean(|xr|, axis=(0, 2))            # over the WHOLE batch
    i0, i1 = top2(activities)                        # descending
    h = concat(xr[:, i0] @ Wc[i0] + bc[i0], xr[:, i1] @ Wc[i1] + bc[i1])
    out = h @ W_final + b_final

Distribution: data-parallel over the batch across 8 NeuronCores (512 rows
each). Per-chunk |x| partial sums are AllReduced (tiny [1,8] collective) so
every core computes the identical top-2 routing.

Perf notes (vs the first working version, 190us; measured ~183-189us, the
collective AllReduce machinery (~36us for 32 bytes) plus core launch skew
dominates and varies +-5us run to run):
  - DMA traffic is spread across the two HWDGE queues (sync/scalar) with
    big descriptors: x as 4 two-chunk slabs [128, 8192] (8KB rows),
    W_final as 8 slabs [128, 4096] (8KB rows), out as [128, 4096] bf16
    row blocks.  The v1 1KB/2KB packets were descriptor-rate bound
    (~10.5ns/packet floor -> one queue at ~121GB/s).
  - W_final (8MB) is NOT loaded during the AllReduce window: a tiny
    cc_out-read DMA on each queue acts as a FIFO blocker so the bus stays
    quiet while the collective's own SDMA work runs (heavy traffic there
    measurably inflates the collective by 2-3x); the slabs then load
    post-AR, fully hidden behind the gather+L1 phase.
  - The routing gathers use a 4-rows-packed DRAM view ([1024, 2048]; one
    row = 4KB) for both x and W_chunks.  Matmul contracts over partitions
    permutation-invariantly, so gathering rows sel*128+p (features
    interleaved 4p+j) works as k-tiles j=0..3 provided BOTH x and W use
    the same row packing.  4x fewer indirect descriptors, 4x bigger.
  - Activity partials are accumulated per partition-half (two chunks per
    slab) and partition-reduced with a half-selector matmul into [2, 4]
    = chunks (0-3; 4-7) flat, which is what the AllReduce sees.
  - Throwaway act_g-gated matmuls re-warm the PE (HAM throttles it cold
    during the AllReduce idle) while gather descriptors generate.
  - bf16 output (halves out bytes; host upcasts; tolerance is 2e-2).
  - L2 loops bt-outer with all 8 PSUM banks so each batch row-block
    finishes contiguously and DMAs out as one 1MB transfer.

Dead ends measured and abandoned: fp8 L2 (rel err 3-4e-2 > 2e-2 gate);
remote_dma_broadcast instead of the collective (this runtime's ucode only
delivers cross-die slots, partially, with a permuted slot map); computing
L1 for all 8 chunks pre-routing with an hT DRAM roundtrip (the extra 6MB
of DMA in the collective window inflated the AllReduce to ~100us).
"""

import numpy as np
import ml_dtypes

import concourse.bass as bass
import concourse.bacc as bacc
import concourse.mybir as mybir
from concourse.tile import TileContext
from concourse.bass_utils import run_bass_kernel_spmd
from concourse.masks import make_identity

dt = mybir.dt
P = 128

NUM_CHUNKS = 8
TOP_K = 2
IN_F = 4096
HID_F = 4096
OUT_F = 4096
B = 4096
CIN = IN_F // NUM_CHUNKS      # 512
COUT = HID_F // NUM_CHUNKS    # 512
N_CORES = 8
BS = B // N_CORES             # 512 rows per core

BT = BS // P                  # 4 batch tiles per core
DT_ = COUT // P               # 4 d-tiles per selected chunk
KF = TOP_K * DT_              # 8 k-tiles for the final matmul
OT = OUT_F // 512             # 8 output column tiles of 512
Q = 4                         # DRAM row packing for activity/gather views

_cache = {}


def _build():
    nc = bacc.Bacc(num_devices=N_CORES, name="chunk_routed_net",
                   num_swdge_queues=4)

    xT = nc.dram_tensor("xT_shard", [IN_F, BS], dt.bfloat16,
                        kind="ExternalInput")
    Wc = nc.dram_tensor("W_chunks", [NUM_CHUNKS, CIN, COUT], dt.bfloat16,
                        kind="ExternalInput")
    bc_t = nc.dram_tensor("b_chunks", [NUM_CHUNKS, COUT], dt.float32,
                          kind="ExternalInput")
    Wf = nc.dram_tensor("W_final", [COUT * TOP_K, OUT_F], dt.bfloat16,
                        kind="ExternalInput")
    bf = nc.dram_tensor("b_final", [1, OUT_F], dt.float32, kind="ExternalInput")
    out = nc.dram_tensor("out_shard", [BS, OUT_F], dt.bfloat16,
                         kind="ExternalOutput")

    # 4-rows-packed views: one row = 4 consecutive 1KB rows = 4KB contiguous.
    # xT_w row r holds xT feature rows 4r..4r+3; chunk c = rows c*128..+128.
    xT_w = xT[:].rearrange("(r q) b -> r (q b)", q=Q)            # [1024, 2048]
    Wc_w = Wc[:].rearrange("a (r q) c -> (a r) (q c)", q=Q)      # [1024, 2048]

    with TileContext(nc) as tc:
        with tc.tile_pool(name="consts", bufs=1) as consts, \
             tc.tile_pool(name="route", bufs=1) as route, \
             tc.tile_pool(name="bfinp", bufs=1) as bfinp, \
             tc.tile_pool(name="wfs", bufs=8) as wfs, \
             tc.tile_pool(name="dram", bufs=1, space="DRAM") as dram:

            # ---------------- constants ----------------
            ones_col = consts.tile([P, 1], dt.float32)     # partition reduce
            nc.vector.memset(ones_col[:], 1.0)
            ones_k1 = consts.tile([1, P], dt.float32)      # K=1 bcast matmul
            nc.vector.memset(ones_k1[:], 1.0)
            ones_k1h = consts.tile([1, P], dt.bfloat16)    # K=1 bf16 bcast
            nc.vector.memset(ones_k1h[:], 1.0)
            ident = consts.tile([P, P], dt.float32)
            make_identity(nc, ident)
            # C_P[p, 0] = p  (row offset within a gather view chunk)
            C_P = consts.tile([P, 1], dt.int32)
            nc.gpsimd.iota(C_P[:], pattern=[[P, 1]], base=0,
                           channel_multiplier=1)
            C_Pf = consts.tile([P, 1], dt.float32)
            nc.vector.tensor_copy(C_Pf[:], C_P[:])
            # C8[p, c] = c  (chunk-id iota along free dim)
            C8 = consts.tile([P, NUM_CHUNKS], dt.int32)
            nc.gpsimd.iota(C8[:], pattern=[[1, NUM_CHUNKS]], base=0,
                           channel_multiplier=0)
            C8f = consts.tile([P, NUM_CHUNKS], dt.float32)
            nc.vector.tensor_copy(C8f[:], C8[:])
            # sel2[p, 0] = (p < 64), sel2[p, 1] = (p >= 64): partition-half
            # selector for the two-chunk activity slabs' partition reduce
            sel2 = consts.tile([P, 2], dt.float32)
            nc.vector.memset(sel2[:], 0.0)
            nc.vector.memset(sel2[0:64, 0:1], 1.0)
            nc.vector.memset(sel2[64:P, 1:2], 1.0)

            with tc.tile_pool(name="ps_early", bufs=1, space="PSUM") as ps_early:
                # ------------ x load + activities ------------
                # 4 two-chunk slabs [128, 8192] from the 8-rows-packed view
                # (8KB descriptors -> half the packet count of 4KB ones; the
                # x load gates the AllReduce trigger, and the slowest cores
                # see ~half DMA bandwidth).  Slab t holds chunk t on
                # partitions 0-63 and chunk t+4 on 64-127; tensor_reduce
                # gives per-partition partials, and the sel2 matmul reduces
                # the halves separately into a [2, 4] = chunks (0-3; 4-7).
                xT_w8 = xT[:].rearrange("(r q) b -> r (q b)", q=8)  # [512, 4096]
                actcol = route.tile([P, 4], dt.float32)
                with tc.tile_pool(name="xl", bufs=1) as xl_pool:
                    xls = []
                    for t in range(4):
                        xlt = xl_pool.tile([P, 8 * BS], dt.bfloat16,
                                           tag=f"xl{t}", name=f"xl{t}")
                        eng = nc.sync if t % 2 == 0 else nc.scalar
                        eng.dma_start(xlt[0:64, :],
                                      xT_w8[t * 64:(t + 1) * 64, :])
                        eng.dma_start(xlt[64:P, :],
                                      xT_w8[(t + 4) * 64:(t + 5) * 64, :])
                        xls.append(xlt)
                    for t in range(4):
                        nc.vector.tensor_reduce(
                            actcol[:, t:t + 1], xls[t][:],
                            axis=mybir.AxisListType.X, op=mybir.AluOpType.add,
                            apply_absolute_value=True)
                    act_ps = ps_early.tile([2, 4], dt.float32, tag="psa")
                    nc.tensor.matmul(act_ps[:], sel2[:], actcol[:],
                                     start=True, stop=True)
                    act_l = route.tile([2, 4], dt.float32)
                    nc.scalar.copy(act_l[:], act_ps[:])

                    # ---- AllReduce (trigger via scalar queue) ----
                    # cc_in is [2, 4] (row-major = chunks 0..7 flat); the
                    # output is read back as [1, 8] — same 32 bytes.
                    cc_in = dram.tile([2, 4], dt.float32)
                    cc_out = dram.tile([1, NUM_CHUNKS], dt.float32)
                    nc.scalar.dma_start(cc_in[:], act_l[:])
                    nc.gpsimd.collective_compute(
                        "AllReduce", mybir.AluOpType.add,
                        replica_groups=[list(range(N_CORES))],
                        ins=[cc_in.opt()], outs=[cc_out.opt()])

                    # ---- tiny prep loads (before the queue blockers) ----
                    bfin = bfinp.tile([1, OUT_F], dt.float32)
                    nc.scalar.dma_start(bfin[:], bf[:])
                    bfin_h = bfinp.tile([1, OUT_F], dt.bfloat16)
                    nc.vector.tensor_copy(bfin_h[:], bfin[:])
                    bfin_bc = bfinp.tile([P, OUT_F], dt.float32)
                    for o in range(OT):
                        sl = slice(o * 512, (o + 1) * 512)
                        ps_b = ps_early.tile([P, 512], dt.float32, tag="psb")
                        nc.tensor.matmul(ps_b[:], ones_k1h[:], bfin_h[:, sl],
                                         start=True, stop=True)
                        nc.vector.tensor_copy(bfin_bc[:, sl], ps_b[:])
                    b_sb = route.tile([NUM_CHUNKS, COUT], dt.float32)
                    nc.scalar.dma_start(b_sb[:], bc_t[:])
                    bT = route.tile([P, DT_ * NUM_CHUNKS], dt.float32)
                    for d in range(DT_):
                        ps_t = ps_early.tile([P, NUM_CHUNKS], dt.float32,
                                             tag="pst")
                        nc.tensor.transpose(
                            ps_t[:], b_sb[:, d * P:(d + 1) * P],
                            ident[0:NUM_CHUNKS, 0:NUM_CHUNKS])
                        nc.scalar.copy(
                            bT[:, d * NUM_CHUNKS:(d + 1) * NUM_CHUNKS],
                            ps_t[:])

                    # ---- quiet bus during the AllReduce ----
                    # v5 showed heavy DMA traffic in the collective window
                    # inflates its machinery by 2-3x (the CC rows share the
                    # 16 SDMA engines and HBM bandwidth).  So W_final is NOT
                    # loaded here: a tiny cc_out read on each HWDGE queue
                    # acts as a FIFO blocker (it waits on the AllReduce), and
                    # the 8MB of W_final slabs queue up behind it, loading
                    # only after the collective completes.  They finish well
                    # inside the post-AR gather+L1 phase, so nothing stalls.
                    blk0 = route.tile([1, NUM_CHUNKS], dt.float32)
                    blk1 = route.tile([1, NUM_CHUNKS], dt.float32)
                    nc.sync.dma_start(blk0[:], cc_out[:])
                    nc.scalar.dma_start(blk1[:], cc_out[:])
                    wf_tiles = []
                    for kf in range(KF):
                        w = wfs.tile([P, OUT_F], dt.bfloat16, tag="wf",
                                     name=f"wf{kf}")
                        eng = nc.sync if kf % 2 == 0 else nc.scalar
                        eng.dma_start(w[:], Wf[kf * P:(kf + 1) * P, :])
                        wf_tiles.append(w)

                    act_g = route.tile([1, NUM_CHUNKS], dt.float32)
                    nc.gpsimd.dma_start(act_g[:], cc_out[:])

                    # ------------ top-2 ------------
                    maxv = route.tile([1, NUM_CHUNKS], dt.float32)
                    maxi = route.tile([1, NUM_CHUNKS], dt.uint32)
                    nc.vector.max(maxv[:], act_g[:])
                    nc.vector.max_index(maxi[:], maxv[:], act_g[:])
                    maxi_f = route.tile([1, NUM_CHUNKS], dt.float32)
                    nc.vector.tensor_copy(maxi_f[:], maxi[:])

                    # bcast[p, j] = idx[j] on every partition (K=1 matmul)
                    bc_ps = ps_early.tile([P, NUM_CHUNKS], dt.float32,
                                          tag="psc")
                    nc.tensor.matmul(bc_ps[:], ones_k1[:], maxi_f[:],
                                     start=True, stop=True)
                    bcast = route.tile([P, NUM_CHUNKS], dt.float32)
                    nc.vector.tensor_copy(bcast[:], bc_ps[:])

                    # ---- HAM warm-up ----
                    # The PE idles ~30-60us during the AllReduce, so the HW
                    # activity manager throttles it back to cold clocks and
                    # L1/L2 would start at ~half rate for ~3.4us.  While the
                    # gather descriptors are being generated (~7us, SWDGE),
                    # run throwaway N=512 matmuls whose rhs depends on the
                    # AllReduce output, keeping them in this idle window.
                    warm_rhs = route.tile([1, 512], dt.bfloat16)
                    nc.vector.tensor_scalar(
                        warm_rhs[:], bfin_h[0:1, 0:512],
                        act_g[0:1, 0:1], scalar2=None,
                        op0=mybir.AluOpType.add)
                    for wi in range(14):
                        ps_w = ps_early.tile([P, 512], dt.float32, tag="psw")
                        nc.tensor.matmul(ps_w[:], ones_k1h[:], warm_rhs[:],
                                         start=True, stop=True)

            # gather offsets into the packed views: off[p, s] = sel_s*128 + p
            bc128 = route.tile([P, TOP_K], dt.float32)
            nc.vector.tensor_scalar_mul(bc128[:], bcast[:, 0:TOP_K], 128.0)
            off_f = route.tile([P, TOP_K], dt.float32)
            for s in range(TOP_K):
                nc.vector.tensor_scalar(
                    off_f[:, s:s + 1], C_Pf[:], bc128[:, s:s + 1],
                    scalar2=None, op0=mybir.AluOpType.add)
            off = route.tile([P, TOP_K], dt.int32)
            nc.vector.tensor_copy(off[:], off_f[:])

            with tc.tile_pool(name="gath", bufs=1) as gath, \
                 tc.tile_pool(name="hts", bufs=1) as hts, \
                 tc.tile_pool(name="outs", bufs=2) as outs:
                # ------------ gathers: [128, 2048] each, 4KB descriptors ---
                # xg4[s][:, j*512:(j+1)*512] = x rows {sel_s*512 + 4p + j}
                # Wg4[s][:, j*512 + m]       = Wc[sel_s] rows {4p + j}, col m
                # (same row permutation => matmul contraction is correct)
                xg4 = [gath.tile([P, Q * BS], dt.bfloat16, tag=f"xg{s}",
                                 name=f"xg{s}") for s in range(TOP_K)]
                Wg4 = [gath.tile([P, Q * COUT], dt.bfloat16, tag=f"wk{s}",
                                 name=f"wk{s}") for s in range(TOP_K)]
                for s in range(TOP_K):
                    nc.gpsimd.indirect_dma_start(
                        out=xg4[s][:], out_offset=None,
                        in_=xT_w,
                        in_offset=bass.IndirectOffsetOnAxis(
                            ap=off[:, s:s + 1], axis=0))
                    nc.gpsimd.indirect_dma_start(
                        out=Wg4[s][:], out_offset=None,
                        in_=Wc_w,
                        in_offset=bass.IndirectOffsetOnAxis(
                            ap=off[:, s:s + 1], axis=0))

                # chunk-bias select: bias[s][d][p] = bT[p, d*8 + sel_s]
                onehot = route.tile([P, TOP_K * NUM_CHUNKS], dt.float32)
                for s in range(TOP_K):
                    nc.vector.tensor_scalar(
                        onehot[:, s * NUM_CHUNKS:(s + 1) * NUM_CHUNKS],
                        C8f[:], bcast[:, s:s + 1], scalar2=None,
                        op0=mybir.AluOpType.is_equal)
                bsel = [[route.tile([P, 1], dt.float32, tag=f"bs{s}_{d}",
                                    name=f"bs{s}_{d}")
                         for d in range(DT_)] for s in range(TOP_K)]
                btmp = route.tile([P, NUM_CHUNKS], dt.float32)
                for s in range(TOP_K):
                    for d in range(DT_):
                        nc.vector.tensor_tensor(
                            out=btmp[:],
                            in0=bT[:, d * NUM_CHUNKS:(d + 1) * NUM_CHUNKS],
                            in1=onehot[:, s * NUM_CHUNKS:(s + 1) * NUM_CHUNKS],
                            op=mybir.AluOpType.mult)
                        nc.vector.tensor_reduce(
                            bsel[s][d][:], btmp[:], axis=mybir.AxisListType.X,
                            op=mybir.AluOpType.add)

                # ------------ L1: hT[s][d] = (x_sel @ Wc[sel]).T + b -------
                hT = [[hts.tile([P, BS], dt.bfloat16, tag=f"ht{s}_{d}",
                                name=f"ht{s}_{d}")
                       for d in range(DT_)] for s in range(TOP_K)]
                with tc.tile_pool(name="ps_h", bufs=2, space="PSUM") as ps_h:
                    for s in range(TOP_K):
                        for d in range(DT_):
                            ph = ps_h.tile([P, BS], dt.float32, tag="ph",
                                           name=f"ph{s}_{d}")
                            for j in range(Q):
                                nc.tensor.matmul(
                                    ph[:],
                                    Wg4[s][:, j * COUT + d * P:
                                           j * COUT + (d + 1) * P],
                                    xg4[s][:, j * BS:(j + 1) * BS],
                                    start=(j == 0), stop=(j == Q - 1))
                            nc.scalar.activation(
                                hT[s][d][:], ph[:],
                                mybir.ActivationFunctionType.Identity,
                                bias=bsel[s][d][:, 0:1])

                # ------------ L2: out = h @ W_final + b_final --------------
                # bt-outer, all 8 PSUM banks per row block; bf16 row block
                # DMAs out on the scalar queue as each completes.
                with tc.tile_pool(name="ps_o", bufs=8, space="PSUM") as ps_o:
                    for bt in range(BT):
                        orow = outs.tile([P, OUT_F], dt.bfloat16, tag="orow",
                                         name=f"orow{bt}")
                        for o in range(OT):
                            osl = slice(o * 512, (o + 1) * 512)
                            po = ps_o.tile([P, 512], dt.float32, tag="po",
                                           name=f"po{bt}_{o}")
                            for kf in range(KF):
                                s, d = divmod(kf, DT_)
                                nc.tensor.matmul(
                                    po[:], hT[s][d][:, bt * P:(bt + 1) * P],
                                    wf_tiles[kf][:, osl],
                                    start=(kf == 0), stop=(kf == KF - 1))
                            nc.vector.tensor_tensor(
                                out=orow[:, osl], in0=po[:],
                                in1=bfin_bc[:, osl],
                                op=mybir.AluOpType.add)
                        nc.scalar.dma_start(
                            out[bt * P:(bt + 1) * P, :], orow[:])
    nc.compile()
    return nc


def kernel(x, W_chunks, b_chunks, W_final, b_final):
    bf16 = ml_dtypes.bfloat16
    x = np.asarray(x, dtype=np.float32).astype(bf16)
    W_chunks = np.asarray(W_chunks, dtype=np.float32).astype(bf16)
    W_final = np.asarray(W_final, dtype=np.float32).astype(bf16)
    b_chunks = np.ascontiguousarray(np.asarray(b_chunks, dtype=np.float32))
    b_final = np.ascontiguousarray(
        np.asarray(b_final, dtype=np.float32).reshape(1, OUT_F))

    if "nc" not in _cache:
        _cache["nc"] = _build()
    nc = _cache["nc"]

    in_maps = [{
        "xT_shard": np.ascontiguousarray(x[c * BS:(c + 1) * BS].T),
        "W_chunks": W_chunks,
        "b_chunks": b_chunks,
        "W_final": W_final,
        "b_final": b_final,
    } for c in range(N_CORES)]

    res = run_bass_kernel_spmd(nc, in_maps, core_ids=list(range(N_CORES)))
    kernel.last_result = res
    return np.concatenate(
        [res.results[c]["out_shard"].astype(np.float32)
         for c in range(N_CORES)], axis=0)


kernel.last_result = None

